# revision 1
# baseline (speedup 1.0000x reference)
"""Multi-head attention (B=2, S=2048, D=1024, H=16) on 8 TRN2 NeuronCores.

Sharding: data-parallel over batch (2) x tensor-parallel over head groups (4).
Core c handles batch c//4, heads [4*(c%4), 4*(c%4)+4).  Each core computes its
heads' attention plus its slice of the output projection (Wo row-slice); the
final all-reduce over head groups happens during the host-side gather-sum.

Per-core device pipeline (matmul operands fp16, accumulation fp32):
  x^T via batched xbar DMA transposes -> Q^T/K^T ([256,2048], head dims on
  partitions) and V ([2048, 4*65] with a ones column per head) -> scores^T =
  K @ Q^T per head (head pairs row-packed on the PE at K=64) -> exp on ScalarE
  (scale=1/8, bias=-12 for fp16 range) -> PV with V_aug stationary, whose ones
  column also accumulates the softmax denominators -> normalize (denominator
  *256 broadcast via PE ones-matmul, reciprocal_approx_fast) -> out^T.T @ Wo.
Projection matmul chains are interleaved between attention groups to keep the
PE dense (HAM stays at full clock).
"""

import numpy as np

import concourse.bass as bass
import concourse.mybir as mybir
from concourse import bacc
from concourse.tile import TileContext
from concourse.bass_utils import run_bass_kernel_spmd

P = 128
S = 2048
D = 1024
H = 16
HD = 64
B = 2
NCORES = 8
HGROUPS = 4
HC = H // HGROUPS          # 4 heads per core
DC = HC * HD               # 256-wide weight slice per core
NST = S // P               # 16 s-tiles (= k-tiles inside attention)
NKT = D // P               # 8 contraction tiles for the projections
QCW = 512
NQC = S // QCW             # 4 q-chunks
VW = HD + 1                # V block width incl. ones column

F32 = mybir.dt.float32
F16 = mybir.dt.float16
AF = mybir.ActivationFunctionType
OP = mybir.AluOpType
EXP_BIAS = -12.0           # keeps exp(q.k/8 - 12) inside fp16 range


def _build(mode):
    """mode: 'causal' | 'allones' | 'general'."""
    nc = bacc.Bacc("TRN2", debug=False, num_devices=NCORES,
                   num_swdge_queues=4)

    xt_in = nc.dram_tensor("xt", [P, NKT, S], F16, kind="ExternalInput")
    wq = nc.dram_tensor("wq", [D, DC], F16, kind="ExternalInput")
    wk = nc.dram_tensor("wk", [D, DC], F16, kind="ExternalInput")
    wv = nc.dram_tensor("wv", [D, DC], F16, kind="ExternalInput")
    wo = nc.dram_tensor("wo", [DC, D], F16, kind="ExternalInput")
    bq = nc.dram_tensor("bq", [DC], F32, kind="ExternalInput")
    bk = nc.dram_tensor("bk", [DC], F32, kind="ExternalInput")
    bv = nc.dram_tensor("bv", [DC], F32, kind="ExternalInput")
    mmast = None
    maskt = None
    if mode == "causal":
        mmast = nc.dram_tensor("mmast", [P, 896], F16, kind="ExternalInput")
    elif mode == "general":
        maskt = nc.dram_tensor("maskt", [NST, P, S], F16, kind="ExternalInput")
    # partials are gather-summed on the host; fp16 halves the output DMA
    out = nc.dram_tensor("out", [S, D], F16, kind="ExternalOutput")

    def nvalid_of(qc):
        return 4 * (qc + 1) if mode == "causal" else NST

    with TileContext(nc) as tc:
        with tc.tile_pool(name="big", bufs=1) as big:
            # warmup operand first, on gpsimd: nothing else queues there at
            # t=0, so the PE warmup matmuls can start immediately
            warm = big.tile([1, QCW], F16, tag="warm", name="warm")
            nc.gpsimd.memset(warm[:], 1.0)
            # ---------- constants / biases ----------
            ones16 = big.tile([1, P], F16, tag="ones16", name="ones16")
            nc.vector.memset(ones16[:], 1.0)
            ebias = big.tile([P, 1], F32, tag="ebias", name="ebias")
            nc.vector.memset(ebias[:], EXP_BIAS)
            bq32 = big.tile([P, 2], F32, tag="bq32", name="bq32")
            bk32 = big.tile([P, 2], F32, tag="bk32", name="bk32")
            bv32 = big.tile([1, DC], F32, tag="bv32", name="bv32")
            bv16 = big.tile([1, DC], F16, tag="bv16", name="bv16")
            mast16 = None
            if mode == "causal":
                mast16 = big.tile([P, 896], F16, tag="mast16", name="mast16")

            # ---------- persistent fp16 operands ----------
            xT = big.tile([P, NKT, S], F16, tag="xT", name="xT")
            QT = [big.tile([P, S], F16, tag=f"QT{m}", name=f"QT{m}")
                  for m in range(2)]
            # K^T stored zero-padded to full 128 contraction rows per head:
            # KTp[mb][0] rows 64:128 are zero, KTp[mb][1] rows 0:64 are zero,
            # so score matmuls contract over 128 partitions (enables FWL).
            KTp = [[big.tile([P, S], F16, tag=f"KTp{m}{h}", name=f"KTp{m}{h}")
                    for h in range(2)] for m in range(2)]
            for m in range(2):
                nc.vector.memset(KTp[m][0][64:128, :], 0.0)
                nc.vector.memset(KTp[m][1][0:64, :], 0.0)
            V = [big.tile([P, HC, VW], F16, tag=f"V{st}", name=f"V{st}")
                 for st in range(NST)]
            outT = [big.tile([P, S], F16, tag=f"outT{m}", name=f"outT{m}")
                    for m in range(2)]
            wq16 = big.tile([P, NKT, DC], F16, tag="wq16", name="wq16")
            wk16 = big.tile([P, NKT, DC], F16, tag="wk16", name="wk16")
            wv16 = big.tile([P, NKT, DC], F16, tag="wv16", name="wv16")
            wo16 = big.tile([P, 2, D], F16, tag="wo16", name="wo16")

            with nc.named_scope("prep"):
                # load the Exp table while DMAs run
                wact = big.tile([1, QCW], F16, tag="wact", name="wact")
                nc.scalar.activation(wact[:], warm[:], AF.Exp,
                                     bias=ebias[0:1, :], scale=0.125)
                # First-needed inputs first, weights spread over all three
                # DMA-issuing engines ahead of the bulk x^T quarters.
                wqr = wq.ap().rearrange("(t p) c -> p t c", p=P)
                wkr = wk.ap().rearrange("(t p) c -> p t c", p=P)
                wvr = wv.ap().rearrange("(t p) c -> p t c", p=P)
                wor = wo.ap().rearrange("(t p) c -> p t c", p=P)
                h0, h1 = slice(0, 4), slice(4, 8)
                HQ = QCW // 2
                nc.sync.dma_start(xT[:, :, 0:HQ], xt_in[:, :, 0:HQ])
                nc.scalar.dma_start(xT[:, :, HQ:QCW], xt_in[:, :, HQ:QCW])
                nc.gpsimd.dma_start(wq16[:, h0, :], wqr[:, h0, :])
                nc.gpsimd.dma_start(wq16[:, h1, :], wqr[:, h1, :])
                nc.gpsimd.dma_start(wk16[:, h0, :], wkr[:, h0, :])
                nc.gpsimd.dma_start(wk16[:, h1, :], wkr[:, h1, :])
                nc.sync.dma_start(xT[:, :, QCW:QCW + HQ],
                                  xt_in[:, :, QCW:QCW + HQ])
                nc.scalar.dma_start(xT[:, :, QCW + HQ:2 * QCW],
                                    xt_in[:, :, QCW + HQ:2 * QCW])
                nc.gpsimd.dma_start(wv16[:, h0, :], wvr[:, h0, :])
                nc.gpsimd.dma_start(wv16[:, h1, :], wvr[:, h1, :])
                # small bias/mask loads kept off the sync/scalar queues so the
                # first x^T chunks land as early as possible
                nc.gpsimd.dma_start(bq32[:],
                                    bq.ap().rearrange("(o p) -> p o", p=P))
                nc.gpsimd.dma_start(bk32[:],
                                    bk.ap().rearrange("(o p) -> p o", p=P))
                nc.gpsimd.dma_start(bv32[:], bv.ap().unsqueeze(0))
                nc.vector.tensor_copy(bv16[:], bv32[:])
                if mode == "causal":
                    nc.gpsimd.dma_start(mast16[:], mmast[:])
                nc.sync.dma_start(xT[:, :, 2 * QCW:3 * QCW],
                                  xt_in[:, :, 2 * QCW:3 * QCW])
                nc.scalar.dma_start(xT[:, :, 3 * QCW:S],
                                    xt_in[:, :, 3 * QCW:S])
                nc.gpsimd.dma_start(wo16[:, 0, :], wor[:, 0, :])
                nc.gpsimd.dma_start(wo16[:, 1, :], wor[:, 1, :])

            # ---------- attention + interleaved projections ----------
            with nc.named_scope("main"), \
                 tc.tile_pool(name="sx0", bufs=1, space="PSUM") as sxp0, \
                 tc.tile_pool(name="sx1", bufs=1, space="PSUM") as sxp1, \
                 tc.tile_pool(name="oa", bufs=1, space="PSUM") as oap, \
                 tc.tile_pool(name="ob", bufs=1, space="PSUM") as obp, \
                 tc.tile_pool(name="pj", bufs=2, space="PSUM") as pjp, \
                 tc.tile_pool(name="pp16", bufs=4) as pp16, \
                 tc.tile_pool(name="nrm", bufs=2) as nrm, \
                 tc.tile_pool(name="ost", bufs=3) as ost, \
                 tc.tile_pool(name="mt", bufs=1) as mtp:

                def qk_chain(mb, qc, warmups=0):
                    qs = slice(qc * QCW, (qc + 1) * QCW)
                    for wi, (w16, bcol) in enumerate(
                            ((wq16, bq32), (wk16, bk32))):
                        ps = pjp.tile([P, QCW], F32, tag="pj", name="pj")
                        for w in range(warmups if wi == 0 else 0):
                            nc.tensor.matmul(ps[:], warm[:, 0:P], warm[:],
                                             start=True, stop=True)
                        for kt in range(NKT):
                            nc.tensor.matmul(
                                ps[:], w16[:, kt, mb * P:(mb + 1) * P],
                                xT[:, kt, qs],
                                start=(kt == 0), stop=(kt == NKT - 1))
                        if wi == 0:
                            nc.vector.tensor_scalar_add(
                                QT[mb][:, qs], ps[:], bcol[:, mb:mb + 1])
                        else:
                            nc.vector.tensor_scalar_add(
                                KTp[mb][0][0:64, qs], ps[0:64, :],
                                bcol[0:64, mb:mb + 1])
                            nc.vector.tensor_scalar_add(
                                KTp[mb][1][64:128, qs], ps[64:128, :],
                                bcol[64:128, mb:mb + 1])

                def v_chain(st):
                    ps = pjp.tile([P, QCW], F32, tag="pj", name="pj")
                    pv = ps[:, 0:DC]
                    for kt in range(NKT):
                        nc.tensor.matmul(
                            pv, xT[:, kt, st * P:(st + 1) * P], wv16[:, kt, :],
                            start=(kt == 0), stop=False)
                    nc.tensor.matmul(pv, ones16[:], bv16[:],
                                     start=False, stop=True)
                    nc.vector.memset(V[st][:], 1.0)
                    nc.vector.tensor_copy(
                        V[st][:, :, 0:HD],
                        ps[:, 0:DC].rearrange("p (h d) -> p h d", h=HC))

                # head start: only what attention-hp0 qc0 needs
                import functools
                with nc.named_scope("proj0"):
                    qk_chain(0, 0, warmups=12)
                    for st in range(4):
                        v_chain(st)

                # filler queue: emitted between attention groups, paced so
                # dependencies are ready ahead of their consumers
                pending = []
                pending += [functools.partial(qk_chain, 0, 1)]
                pending += [functools.partial(v_chain, st) for st in (4, 5)]
                pending += [functools.partial(qk_chain, 0, 2)]
                pending += [functools.partial(v_chain, st) for st in (6, 7)]
                pending += [functools.partial(qk_chain, 0, 3)]
                pending += [functools.partial(v_chain, st)
                            for st in range(8, 16)]
                pending += [functools.partial(qk_chain, 1, qc)
                            for qc in range(NQC)]

                def d_chunk(qb, nh):
                    ns = slice(nh * QCW, (nh + 1) * QCW)
                    ps = pjp.tile([P, QCW], F32, tag="pj", name="pj")
                    for t in range(2):
                        nc.tensor.matmul(
                            ps[:], outT[t][:, qb * P:(qb + 1) * P],
                            wo16[:, t, ns], start=(t == 0), stop=(t == 1))
                    ob = ost.tile([P, QCW], F16, tag="ob", name="ob")
                    nc.vector.tensor_copy(ob[:], ps[:])
                    oeng = (nc.sync, nc.scalar, nc.gpsimd)[(2 * qb + nh) % 3]
                    oeng.dma_start(out[qb * P:(qb + 1) * P, ns], ob[:])

                def qc_done(hp, qc):
                    if hp == 1:
                        for qb in range(4 * qc, 4 * qc + 4):
                            pending.append(functools.partial(d_chunk, qb, 0))
                            pending.append(functools.partial(d_chunk, qb, 1))

                def attention(hp, interleave):
                    hA, hB = 2 * hp, 2 * hp + 1
                    maskt_sb = {}
                    if mode == "general":
                        for kt in range(NST):
                            mts = mtp.tile([P, S], F16, tag=f"mts{kt}",
                                           name=f"mts{kt}")
                            eng = (nc.sync, nc.scalar, nc.gpsimd)[kt % 3]
                            eng.dma_start(mts[:], maskt[kt])
                            maskt_sb[kt] = mts
                    for qc in range(NQC):
                        qs = slice(qc * QCW, (qc + 1) * QCW)
                        nvalid = nvalid_of(qc)
                        oA = oap.tile([VW, QCW], F32, tag="oA", name="oA")
                        oB = obp.tile([VW, QCW], F32, tag="oB", name="oB")
                        for kt in range(nvalid):
                            ks = slice(kt * P, (kt + 1) * P)
                            pool = sxp0 if kt % 2 == 0 else sxp1
                            tg = f"sx{kt % 2}"
                            sx = pool.tile([P, 2 * QCW], F32, tag=tg, name=tg)
                            nc.tensor.matmul(
                                sx[:, 0:QCW],
                                KTp[hp][0][:, ks], QT[hp][:, qs],
                                start=True, stop=True)
                            nc.tensor.matmul(
                                sx[:, QCW:2 * QCW],
                                KTp[hp][1][:, ks], QT[hp][:, qs],
                                start=True, stop=True)
                            p16 = pp16.tile([P, 2 * QCW], F16, tag="p16",
                                            name="p16")
                            nc.scalar.activation(p16[:], sx[:], AF.Exp,
                                                 bias=ebias[:], scale=0.125)
                            msl = None
                            if mode == "causal" and kt >= 4 * qc:
                                t = kt - 4 * qc
                                msl = mast16[:, 384 - P * t:896 - P * t]
                            elif mode == "general":
                                msl = maskt_sb[kt][:, qs]
                            if msl is not None:
                                pv2 = p16[:].rearrange("p (h w) -> p h w", h=2)
                                nc.vector.tensor_tensor(
                                    pv2, pv2,
                                    msl.unsqueeze(1).to_broadcast(
                                        (P, 2, QCW)),
                                    OP.mult)
                            nc.tensor.matmul(
                                oA[:], V[kt][:, hA, :], p16[:, 0:QCW],
                                start=(kt == 0), stop=(kt == nvalid - 1))
                            nc.tensor.matmul(
                                oB[:], V[kt][:, hB, :], p16[:, QCW:2 * QCW],
                                start=(kt == 0), stop=(kt == nvalid - 1))
                            if interleave and pending:
                                pending.pop(0)()
                                if hp == 1 and pending:
                                    pending.pop(0)()
                        # normalize: denom*256 (fp16 normal range), PE ones
                        # broadcast, fp32 fast reciprocal, fused *256 mul
                        for o_ps, row in ((oA, 0), (oB, 64)):
                            den = nrm.tile([1, QCW], F16, tag="den", name="den")
                            with nc.allow_low_precision(
                                    reason="softmax denom in fp16 (scaled)"):
                                nc.vector.tensor_scalar_mul(
                                    den[:], o_ps[HD:VW, :], 256.0)
                            bc_ps = pjp.tile([64, QCW], F32, tag="pj",
                                             name="bc")
                            nc.tensor.matmul(bc_ps[:], ones16[:, 0:64], den[:],
                                             start=True, stop=True)
                            rdb = nrm.tile([64, QCW], F32, tag="rdb",
                                           name="rdb")
                            nc.vector.reciprocal_approx_fast(rdb[:], bc_ps[:])
                            nc.vector.scalar_tensor_tensor(
                                outT[hp][row:row + 64, qs], o_ps[0:HD, :],
                                256.0, rdb[:], OP.mult, OP.mult)
                        if interleave:
                            qc_done(hp, qc)
                    while interleave and pending:
                        pending.pop(0)()

                with nc.named_scope("attn0"):
                    attention(0, True)
                with nc.named_scope("attn1"):
                    attention(1, True)


    nc.compile()
    return nc


_BUILD_CACHE = {}


def _get_module(mode):
    if mode not in _BUILD_CACHE:
        _BUILD_CACHE[mode] = _build(mode)
    return _BUILD_CACHE[mode]


def _causal_master():
    kk = np.arange(P)[:, None]
    w = np.arange(896)[None, :]
    return (kk <= w - 384).astype(np.float16)


def kernel(**inputs):
    x = np.ascontiguousarray(np.asarray(inputs["x"], dtype=np.float32))
    attn_mask = np.asarray(inputs["attn_mask"])
    Wq = np.asarray(inputs["Wq"], dtype=np.float32)
    Wk = np.asarray(inputs["Wk"], dtype=np.float32)
    Wv = np.asarray(inputs["Wv"], dtype=np.float32)
    Wo = np.asarray(inputs["Wo"], dtype=np.float32)
    bq = np.asarray(inputs["bq"], dtype=np.float32)
    bk = np.asarray(inputs["bk"], dtype=np.float32)
    bv = np.asarray(inputs["bv"], dtype=np.float32)
    bo = np.asarray(inputs["bo"], dtype=np.float32)

    m = attn_mask.reshape(B, attn_mask.shape[-2], attn_mask.shape[-1])
    if m.all():
        mode = "allones"
    elif all(np.array_equal(m[b], np.tril(np.ones((S, S), dtype=bool)))
             for b in range(B)):
        mode = "causal"
    else:
        mode = "general"

    nc = _get_module(mode)

    in_maps = []
    for c in range(NCORES):
        b, hg = c // HGROUPS, c % HGROUPS
        cs = slice(hg * DC, (hg + 1) * DC)
        xt = x[b].T.astype(np.float16).reshape(NKT, P, S)
        im = {
            "xt": np.ascontiguousarray(xt.transpose(1, 0, 2)),
            "wq": np.ascontiguousarray(Wq[:, cs].astype(np.float16)),
            "wk": np.ascontiguousarray(Wk[:, cs].astype(np.float16)),
            "wv": np.ascontiguousarray(Wv[:, cs].astype(np.float16)),
            "wo": np.ascontiguousarray(Wo[cs, :].astype(np.float16)),
            "bq": np.ascontiguousarray(bq[cs]),
            "bk": np.ascontiguousarray(bk[cs]),
            "bv": np.ascontiguousarray(bv[cs]),
        }
        if mode == "causal":
            im["mmast"] = _causal_master()
        elif mode == "general":
            im["maskt"] = np.ascontiguousarray(
                m[b].T.astype(np.float16).reshape(NST, P, S))
        in_maps.append(im)

    res = run_bass_kernel_spmd(nc, in_maps, core_ids=list(range(NCORES)))

    out = np.zeros((B, S, D), dtype=np.float32)
    for c in range(NCORES):
        out[c // HGROUPS] += res.results[c]["out"].astype(np.float32)
    out += bo[None, None, :]
    return out

